# revision 44
# baseline (speedup 1.0000x reference)
"""MoE (dense routing) Trainium2 kernel — all-bf16 pipeline.

Math: out = softmax(x@Wg+bg) -weighted sum over experts of
      (gelu(x@W1[e]+b1[e]) @ W2[e] + b2[e]).

Strategy (data-parallel over 8 cores, 2048 tokens each):
  - Every matmul operand is bf16 (1 PE cycle/row at any moving size,
    half the DMA/SBUF traffic of fp32r); PSUM accumulation stays fp32.
    End-to-end bf16 numerics measured at ~5e-3 rel err vs the fp32
    reference.
  - Host pre-lays all operands in the exact SBUF layouts (partition-major)
    so every DMA is a plain strided copy with >=1KB descriptors.
  - Layer 1 runs "transposed": hT[ej, t] accumulated over d-chunks with
    W1 tiles stationary and x chunks moving; T=512 token blocks halve
    the matmul count vs T=256 (less per-instruction overhead).
  - b1 is the ACT bias of the gelu; gelu output lands directly in the
    sh tile (bf16) and the DVE scales it in place by the broadcast
    gate weight (unnormalized exp; 1/sum applied on the output copy).
  - Gate: logitsT[e, t] accumulated like L1; exp fused with +bg on ACT.
    The per-token denominators (K=8 ones-matmuls -> reciprocal) are
    emitted at the start of l2_stage so the PE never waits on the ACT.
  - Layer 2: out[t, o] accumulated over all ej chunks in PSUM, seeded
    with expT @ b2 (start=True) which realizes the sum_e w_e*b2[e] term;
    final ACT copy applies the 1/sum_e exp per-token scale and writes
    bf16 (host upcasts to fp32).
  - Weight prologue: the first ~30us is HBM-read-bound (~330GB/s/core
    aggregate), so only block-0 x, W1 (in consumption order), and x block 1
    ride the three DMA queues early; W2 transfers carry a WAW dependency on
    a tiny ACT write emitted mid-L1(0) so they start only once L1 is fed.
    Exp broadcasts pace naturally on the Pool queue; output DMAs are
    round-robined across all three queues.
No transposes on device at all.
"""

import numpy as np
from contextlib import ExitStack

import ml_dtypes
import orjson

import concourse.bass as bass
import concourse.bass2jax as bass2jax
import concourse.bass_utils as bass_utils
import concourse.tile as tile
from concourse import mybir
from concourse.bass_utils import run_bass_kernel_spmd

# The walrus build in this container rejects any instruction carrying more
# than one sync wait ("Too many sync wait commands", CoreV3GenImpl
# setupSyncWait), but the tile scheduler freely attaches several. Split the
# extras onto standalone single-wait EventSemaphore carriers placed just
# before the instruction (same engine, so program order is preserved).
_orig_compile_bir_kernel = bass_utils.compile_bir_kernel


def _split_multiwait_bir(bir_json):
    bir = orjson.loads(bir_json)
    changed = False
    for fn in bir.get("functions", []):
        for blk in fn.get("blocks", []):
            ins_list = blk.get("instructions")
            if not ins_list:
                continue
            out = []
            for inst in ins_list:
                si = inst.get("sync_info")
                if si:
                    waits = si.get("on_wait") or []
                    if len(waits) > 1:
                        changed = True
                        for k, w in enumerate(waits[:-1]):
                            carrier = {
                                "engine": inst["engine"],
                                "ins": [],
                                "outs": [],
                                "name": f"{inst['name']}_xw{k}",
                                "opcode": "EventSemaphore",
                                "sync_info": {"on_update": [], "on_wait": [w]},
                            }
                            if "debug" in inst:
                                carrier["debug"] = inst["debug"]
                            out.append(carrier)
                        si["on_wait"] = [waits[-1]]
                out.append(inst)
            blk["instructions"] = out
    return orjson.dumps(bir) if changed else bir_json


def _compile_bir_kernel_split(bir_json, tmpdir, neff_name="file.neff"):
    return _orig_compile_bir_kernel(_split_multiwait_bir(bir_json), tmpdir, neff_name)


bass_utils.compile_bir_kernel = _compile_bir_kernel_split
bass2jax.compile_bir_kernel = _compile_bir_kernel_split

N, D, H, O, E = 16384, 1024, 256, 1024, 8
NCORES = 8
NTOK = N // NCORES  # tokens per core
P = 128
T = 512  # token block size
TS = T // P  # 128-token sub-blocks per block
NB = NTOK // T  # token blocks per core
DC = D // P  # d chunks (contraction, layer 1)
EJ = E * H  # packed hidden width
NEJ = EJ // P  # ej chunks (contraction, layer 2)
JC_PER_E = H // P  # ej chunks per expert
OH = O // 2  # layer-2 output half width (one PSUM bank)

FP = mybir.dt.float32
BF = mybir.dt.bfloat16
AF = mybir.ActivationFunctionType
BF_NP = ml_dtypes.bfloat16


def _build_nc():
    nc = bass.Bass()
    xh = nc.dram_tensor("xh", [P, DC, NTOK], BF, kind="ExternalInput")
    W1h = nc.dram_tensor("W1h", [P, EJ, DC], BF, kind="ExternalInput")
    Wgh = nc.dram_tensor("Wgh", [P, DC, E], BF, kind="ExternalInput")
    W2h = nc.dram_tensor("W2h", [P, NEJ, O], BF, kind="ExternalInput")
    b1h = nc.dram_tensor("b1h", [P, NEJ], FP, kind="ExternalInput")
    bgh = nc.dram_tensor("bgh", [E, 1], FP, kind="ExternalInput")
    b2h = nc.dram_tensor("b2h", [E, O], BF, kind="ExternalInput")
    out = nc.dram_tensor("out", [NTOK, O], BF, kind="ExternalOutput")

    with tile.TileContext(nc) as tc, ExitStack() as ctx:
        const = ctx.enter_context(tc.tile_pool(name="const", bufs=1))
        dpool = ctx.enter_context(tc.tile_pool(name="dram", bufs=2, space="DRAM"))
        xpool = ctx.enter_context(tc.tile_pool(name="xts", bufs=3))
        shpool = ctx.enter_context(tc.tile_pool(name="sh", bufs=3))
        bcpool = ctx.enter_context(tc.tile_pool(name="bc", bufs=2))
        epool = ctx.enter_context(tc.tile_pool(name="expp", bufs=3))
        opool = ctx.enter_context(tc.tile_pool(name="outp", bufs=6))
        rpool = ctx.enter_context(tc.tile_pool(name="rcp", bufs=2))
        ps_h = ctx.enter_context(tc.tile_pool(name="ps_h", bufs=2, space="PSUM"))
        ps_g = ctx.enter_context(tc.tile_pool(name="ps_g", bufs=1, space="PSUM"))
        ps_s = ctx.enter_context(tc.tile_pool(name="ps_s", bufs=2, space="PSUM"))
        ps_o = ctx.enter_context(tc.tile_pool(name="ps_o", bufs=2, space="PSUM"))

        Wgs = const.tile([P, DC, E], BF)
        b1s = const.tile([P, NEJ], FP)
        bgs = const.tile([E, 1], FP)
        b2s = const.tile([E, O], BF)
        ones8 = const.tile([E, 1], BF)
        W1s = const.tile([P, EJ, DC], BF)
        W2s = const.tile([P, NEJ, O], BF)

        xtss = {}

        def load_xts(blk, eng):
            t0 = blk * T
            xts = xpool.tile([P, DC, T], BF, name=f"xts{blk}", tag="xts")
            eng.dma_start(xts[:], xh[:, :, t0 : t0 + T])
            xtss[blk] = xts

        # --- prologue DMA schedule ---------------------------------------
        # All three queues (SP + ACT hardware DGE, Pool software DGE) carry
        # weights, in exact PE consumption order. Block-0 x is split per
        # d-chunk across the queues so the first gate matmul can start as
        # soon as chunk dc=0 lands (~4us). The Pool queue takes its W1 share
        # BEFORE the data-dependent exp broadcast (which would block the
        # queue head until the gate ACT finishes).
        QS = [nc.sync, nc.scalar, nc.gpsimd]
        nc.scalar.dma_start(Wgs[:], Wgh[:])
        nc.scalar.dma_start(bgs[:], bgh[:])
        nc.gpsimd.dma_start(b1s[:], b1h[:])
        xts0 = xpool.tile([P, DC, T], BF, name="xts0", tag="xts")
        # gate consumes xts0 per d-chunk in order; interleave so chunk dc
        # lands roughly in consumption order across the three queues
        XQ = [1, 0, 1, 0, 1, 0, 1, 0]
        for dc in range(DC):
            QS[XQ[dc]].dma_start(xts0[:, dc, :], xh[:, dc, 0:T])
        xtss[0] = xts0
        nc.gpsimd.memset(ones8[:], 1.0)
        # W1 ejc chunks, split so the merged 3-queue arrival order tracks the
        # PE's consumption order (1.84us/chunk from ~12us): the Pool queue
        # has no xts0 ahead of it so it delivers c0/c1 first; x block 1 rides
        # the sync queue after its second W1 chunk so gate(1) never waits
        W1Q = [0, 1, 2, 0, 1, 2, 0, 1, 0, 1, 2, 0, 1, 2, 0, 1]
        for c in range(16):
            sl = slice(c * P, (c + 1) * P)
            QS[W1Q[c]].dma_start(W1s[:, sl, :], W1h[:, sl, :])
            if c == 3:
                load_xts(1, nc.sync)
        nc.scalar.dma_start(b2s[:], b2h[:])

        def load_w2():
            # W2 is not consumed until L2(0) (~72us) but 4MB of it racing the
            # prologue starves the HBM pipe (~330GB/s/core aggregate) right
            # when W1/x/bc0 are critical. Pace it: a tiny ACT write into one
            # element of every W2 chunk region (emitted after gelu ejc=11 of
            # block 0, ~28us) gives each W2 DMA a WAW dependency on the ACT,
            # so the transfers only start once L1(0) is mostly fed.
            nc.scalar.activation(
                W2s[0:1, :, 0:1], b1s[0:1, 0:NEJ], AF.Copy, scale=0.0
            )
            for g in range(NEJ):
                QS[g % 3].dma_start(W2s[:, g : g + 1, :], W2h[:, g : g + 1, :])
            load_xts(2, nc.sync)
            load_xts(3, nc.scalar)

        def gate_stage(blk):
            # gate logits (transposed): gt[e, t]; exp kept UNNORMALIZED.
            xts = xtss[blk]
            gt = ps_g.tile([E, T], FP, name=f"gt{blk}", tag="gt")
            for dc in range(DC):
                nc.tensor.matmul(
                    gt[:],
                    Wgs[:, dc, :],
                    xts[:, dc, :],
                    start=(dc == 0),
                    stop=(dc == DC - 1),
                )
            expv = epool.tile([E, T], BF, name=f"exp{blk}", tag="exp")
            nc.scalar.activation(expv[:], gt[:], AF.Exp, bias=bgs[:, 0:1])
            # broadcast exp rows across partitions for the hidden scaling
            # (partition-stride-0 DMA only legal from DRAM -> bounce there)
            expd = dpool.tile([E, T], BF, name=f"expd{blk}", tag="expd")
            nc.gpsimd.dma_start(expd[:], expv[:])
            bc = bcpool.tile([P, E, T], BF, name=f"bc{blk}", tag="bc")
            for e in range(E):
                nc.gpsimd.dma_start(bc[:, e, :], expd[e : e + 1, :].to_broadcast((P, T)))
            return expv, bc

        def l1_stage(blk, bc):
            xts = xtss[blk]
            sh = shpool.tile([P, NEJ, T], BF, name=f"sh{blk}", tag="sh")
            for ejc in range(NEJ):
                ht = ps_h.tile([P, T], FP, name=f"ht{blk}_{ejc}", tag="ht")
                for dc in range(DC):
                    nc.tensor.matmul(
                        ht[:],
                        W1s[:, ejc * P : (ejc + 1) * P, dc],
                        xts[:, dc, :],
                        start=(dc == 0),
                        stop=(dc == DC - 1),
                    )
                nc.scalar.activation(
                    sh[:, ejc, :], ht[:], AF.Gelu, bias=b1s[:, ejc : ejc + 1]
                )
                nc.vector.tensor_tensor(
                    sh[:, ejc, :],
                    sh[:, ejc, :],
                    bc[:, ejc // JC_PER_E, :],
                    mybir.AluOpType.mult,
                )
                if blk == 0 and ejc == 11:
                    load_w2()
            return sh

        OUT_ENG = [nc.sync, nc.scalar, nc.gpsimd]

        def l2_stage(blk, sh, expv):
            t0 = blk * T
            # per-token softmax denominators (landed in token-partition
            # layout via K=8 ones matmuls), then reciprocals on the DVE
            rcp = rpool.tile([P, TS], FP, name=f"rcp{blk}", tag="rcp")
            for ts in range(TS):
                s = ps_s.tile([P, 1], FP, name=f"s{blk}_{ts}", tag="s")
                nc.tensor.matmul(
                    s[:],
                    expv[:, ts * P : (ts + 1) * P],
                    ones8[:],
                    start=True,
                    stop=True,
                )
                nc.vector.reciprocal(rcp[:, ts : ts + 1], s[:])
            for ts in range(TS):
                tsl = slice(ts * P, (ts + 1) * P)
                for half in range(2):
                    o0 = half * OH
                    ops = ps_o.tile([P, OH], FP, name=f"ops{blk}_{ts}_{half}", tag="ops")
                    nc.tensor.matmul(
                        ops[:],
                        expv[:, tsl],
                        b2s[:, o0 : o0 + OH],
                        start=True,
                        stop=False,
                    )
                    for ejc in range(NEJ):
                        nc.tensor.matmul(
                            ops[:],
                            sh[:, ejc, tsl],
                            W2s[:, ejc, o0 : o0 + OH],
                            start=False,
                            stop=(ejc == NEJ - 1),
                        )
                    outsb = opool.tile([P, OH], BF, name=f"o{blk}_{ts}_{half}", tag="o")
                    nc.scalar.activation(
                        outsb[:], ops[:], AF.Copy, scale=rcp[:, ts : ts + 1]
                    )
                    OUT_ENG[(ts * 2 + half) % 3].dma_start(
                        out[t0 + ts * P : t0 + (ts + 1) * P, o0 : o0 + OH], outsb[:]
                    )

        # --- pipeline: gate(b+1) and L2(b-1) interleave with L1(b) -------
        states = {0: gate_stage(0)}
        sh_prev = None
        for blk in range(NB):
            sh = l1_stage(blk, states[blk][1])
            if blk + 1 < NB:
                states[blk + 1] = gate_stage(blk + 1)
            if blk >= 1:
                l2_stage(blk - 1, sh_prev, states[blk - 1][0])
            sh_prev = sh
        l2_stage(NB - 1, sh_prev, states[NB - 1][0])
    return nc


_CACHE = {}


def kernel(**inputs):
    x = np.asarray(inputs["x"], dtype=np.float32)
    W1 = np.asarray(inputs["W1"], dtype=np.float32)
    b1 = np.asarray(inputs["b1"], dtype=np.float32)
    W2 = np.asarray(inputs["W2"], dtype=np.float32)
    b2 = np.asarray(inputs["b2"], dtype=np.float32)
    Wg = np.asarray(inputs["Wg"], dtype=np.float32)
    bg = np.asarray(inputs["bg"], dtype=np.float32)

    # host pre-layouts (all partition-major, bf16 where a matmul consumes it)
    W1p = W1.transpose(1, 0, 2).reshape(D, EJ)  # [d, ej]
    W1h = np.ascontiguousarray(
        W1p.reshape(DC, P, EJ).transpose(1, 2, 0)
    ).astype(BF_NP)  # [p, ej, dc]
    Wgh = np.ascontiguousarray(Wg.reshape(DC, P, E).transpose(1, 0, 2)).astype(BF_NP)
    W2h = np.ascontiguousarray(
        W2.reshape(EJ, O).reshape(NEJ, P, O).transpose(1, 0, 2)
    ).astype(BF_NP)  # [p, ec, o]
    b1h = np.ascontiguousarray(b1.reshape(EJ).reshape(NEJ, P).T)
    bgh = np.ascontiguousarray(bg.reshape(E, 1))
    b2h = b2.astype(BF_NP)

    if "nc" not in _CACHE:
        _CACHE["nc"] = _build_nc()
    nc = _CACHE["nc"]

    x16 = x.astype(BF_NP)
    in_maps = []
    for c in range(NCORES):
        xs = x16[c * NTOK : (c + 1) * NTOK]  # [ntok, d]
        xhc = np.ascontiguousarray(
            xs.T.reshape(DC, P, NTOK).transpose(1, 0, 2)
        )  # [p, dc, t]
        in_maps.append(
            {
                "xh": xhc,
                "W1h": W1h,
                "Wgh": Wgh,
                "W2h": W2h,
                "b1h": b1h,
                "bgh": bgh,
                "b2h": b2h,
            }
        )

    res = run_bass_kernel_spmd(nc, in_maps, list(range(NCORES)))
    kernel.last = res
    return np.concatenate(
        [res.results[c]["out"].astype(np.float32) for c in range(NCORES)], axis=0
    )


# revision 45
# speedup vs baseline: 1.0080x; 1.0080x over previous
"""MoE (dense routing) Trainium2 kernel — all-bf16 pipeline.

Math: out = softmax(x@Wg+bg) -weighted sum over experts of
      (gelu(x@W1[e]+b1[e]) @ W2[e] + b2[e]).

Strategy (data-parallel over 8 cores, 2048 tokens each):
  - Every matmul operand is bf16 (1 PE cycle/row at any moving size,
    half the DMA/SBUF traffic of fp32r); PSUM accumulation stays fp32.
    End-to-end bf16 numerics measured at ~5e-3 rel err vs the fp32
    reference.
  - Host pre-lays all operands in the exact SBUF layouts (partition-major)
    so every DMA is a plain strided copy with >=1KB descriptors.
  - Layer 1 runs "transposed": hT[ej, t] accumulated over d-chunks with
    W1 tiles stationary and x chunks moving; T=512 token blocks halve
    the matmul count vs T=256 (less per-instruction overhead).
  - b1 is the ACT bias of the gelu; gelu output lands directly in the
    sh tile (bf16) and the DVE scales it in place by the broadcast
    gate weight (unnormalized exp; 1/sum applied on the output copy).
  - Gate: logitsT[e, t] accumulated like L1; exp fused with +bg on ACT.
    The per-token denominators (K=8 ones-matmuls -> reciprocal) are
    emitted at the start of l2_stage so the PE never waits on the ACT.
  - Layer 2: out[t, o] accumulated over all ej chunks in PSUM, seeded
    with expT @ b2 (start=True) which realizes the sum_e w_e*b2[e] term;
    final ACT copy applies the 1/sum_e exp per-token scale and writes
    bf16 (host upcasts to fp32).
  - Weight prologue: the first ~30us is HBM-read-bound (~330GB/s/core
    aggregate), so only block-0 x, W1 (in consumption order), and x block 1
    ride the three DMA queues early; W2 transfers carry a WAW dependency on
    a tiny ACT write emitted mid-L1(0) so they start only once L1 is fed.
    Exp broadcasts pace naturally on the Pool queue; output DMAs are
    round-robined across all three queues.
No transposes on device at all.
"""

import numpy as np
from contextlib import ExitStack

import ml_dtypes
import orjson

import concourse.bass as bass
import concourse.bass2jax as bass2jax
import concourse.bass_utils as bass_utils
import concourse.tile as tile
from concourse import mybir
from concourse.bass_utils import run_bass_kernel_spmd

# The walrus build in this container rejects any instruction carrying more
# than one sync wait ("Too many sync wait commands", CoreV3GenImpl
# setupSyncWait), but the tile scheduler freely attaches several. Split the
# extras onto standalone single-wait EventSemaphore carriers placed just
# before the instruction (same engine, so program order is preserved).
_orig_compile_bir_kernel = bass_utils.compile_bir_kernel


def _split_multiwait_bir(bir_json):
    bir = orjson.loads(bir_json)
    changed = False
    for fn in bir.get("functions", []):
        for blk in fn.get("blocks", []):
            ins_list = blk.get("instructions")
            if not ins_list:
                continue
            out = []
            for inst in ins_list:
                si = inst.get("sync_info")
                if si:
                    waits = si.get("on_wait") or []
                    if len(waits) > 1:
                        changed = True
                        for k, w in enumerate(waits[:-1]):
                            carrier = {
                                "engine": inst["engine"],
                                "ins": [],
                                "outs": [],
                                "name": f"{inst['name']}_xw{k}",
                                "opcode": "EventSemaphore",
                                "sync_info": {"on_update": [], "on_wait": [w]},
                            }
                            if "debug" in inst:
                                carrier["debug"] = inst["debug"]
                            out.append(carrier)
                        si["on_wait"] = [waits[-1]]
                out.append(inst)
            blk["instructions"] = out
    return orjson.dumps(bir) if changed else bir_json


def _compile_bir_kernel_split(bir_json, tmpdir, neff_name="file.neff"):
    return _orig_compile_bir_kernel(_split_multiwait_bir(bir_json), tmpdir, neff_name)


bass_utils.compile_bir_kernel = _compile_bir_kernel_split
bass2jax.compile_bir_kernel = _compile_bir_kernel_split

N, D, H, O, E = 16384, 1024, 256, 1024, 8
NCORES = 8
NTOK = N // NCORES  # tokens per core
P = 128
T = 512  # token block size
TS = T // P  # 128-token sub-blocks per block
NB = NTOK // T  # token blocks per core
DC = D // P  # d chunks (contraction, layer 1)
EJ = E * H  # packed hidden width
NEJ = EJ // P  # ej chunks (contraction, layer 2)
JC_PER_E = H // P  # ej chunks per expert
OH = O // 2  # layer-2 output half width (one PSUM bank)

FP = mybir.dt.float32
BF = mybir.dt.bfloat16
AF = mybir.ActivationFunctionType
BF_NP = ml_dtypes.bfloat16


def _build_nc():
    nc = bass.Bass()
    xh = nc.dram_tensor("xh", [P, DC, NTOK], BF, kind="ExternalInput")
    W1h = nc.dram_tensor("W1h", [P, EJ, DC], BF, kind="ExternalInput")
    Wgh = nc.dram_tensor("Wgh", [P, DC, E], BF, kind="ExternalInput")
    W2h = nc.dram_tensor("W2h", [P, NEJ, O], BF, kind="ExternalInput")
    b1h = nc.dram_tensor("b1h", [P, NEJ], FP, kind="ExternalInput")
    bgh = nc.dram_tensor("bgh", [E, 1], FP, kind="ExternalInput")
    b2h = nc.dram_tensor("b2h", [E, O], BF, kind="ExternalInput")
    out = nc.dram_tensor("out", [NTOK, O], BF, kind="ExternalOutput")

    with tile.TileContext(nc) as tc, ExitStack() as ctx:
        const = ctx.enter_context(tc.tile_pool(name="const", bufs=1))
        dpool = ctx.enter_context(tc.tile_pool(name="dram", bufs=2, space="DRAM"))
        xpool = ctx.enter_context(tc.tile_pool(name="xts", bufs=3))
        shpool = ctx.enter_context(tc.tile_pool(name="sh", bufs=2))
        bcpool = ctx.enter_context(tc.tile_pool(name="bc", bufs=2))
        epool = ctx.enter_context(tc.tile_pool(name="expp", bufs=3))
        opool = ctx.enter_context(tc.tile_pool(name="outp", bufs=4))
        rpool = ctx.enter_context(tc.tile_pool(name="rcp", bufs=2))
        ps_h = ctx.enter_context(tc.tile_pool(name="ps_h", bufs=2, space="PSUM"))
        ps_g = ctx.enter_context(tc.tile_pool(name="ps_g", bufs=1, space="PSUM"))
        ps_s = ctx.enter_context(tc.tile_pool(name="ps_s", bufs=2, space="PSUM"))
        ps_o = ctx.enter_context(tc.tile_pool(name="ps_o", bufs=2, space="PSUM"))

        Wgs = const.tile([P, DC, E], BF)
        b1s = const.tile([P, NEJ], FP)
        bgs = const.tile([E, 1], FP)
        b2s = const.tile([E, O], BF)
        ones8 = const.tile([E, 1], BF)
        W1s = const.tile([P, EJ, DC], BF)
        W2s = const.tile([P, NEJ, O], BF)

        xtss = {}

        def load_xts(blk, eng):
            t0 = blk * T
            xts = xpool.tile([P, DC, T], BF, name=f"xts{blk}", tag="xts")
            eng.dma_start(xts[:], xh[:, :, t0 : t0 + T])
            xtss[blk] = xts

        # --- prologue DMA schedule ---------------------------------------
        # All three queues (SP + ACT hardware DGE, Pool software DGE) carry
        # weights, in exact PE consumption order. Block-0 x is split per
        # d-chunk across the queues so the first gate matmul can start as
        # soon as chunk dc=0 lands (~4us). The Pool queue takes its W1 share
        # BEFORE the data-dependent exp broadcast (which would block the
        # queue head until the gate ACT finishes).
        QS = [nc.sync, nc.scalar, nc.gpsimd]
        nc.scalar.dma_start(Wgs[:], Wgh[:])
        nc.scalar.dma_start(bgs[:], bgh[:])
        nc.gpsimd.dma_start(b1s[:], b1h[:])
        xts0 = xpool.tile([P, DC, T], BF, name="xts0", tag="xts")
        # gate consumes xts0 per d-chunk in order; interleave so chunk dc
        # lands roughly in consumption order across the three queues
        XQ = [1, 0, 1, 0, 1, 0, 1, 0]
        for dc in range(DC):
            QS[XQ[dc]].dma_start(xts0[:, dc, :], xh[:, dc, 0:T])
        xtss[0] = xts0
        nc.gpsimd.memset(ones8[:], 1.0)
        # W1 ejc chunks, split so the merged 3-queue arrival order tracks the
        # PE's consumption order (1.84us/chunk from ~12us): the Pool queue
        # has no xts0 ahead of it so it delivers c0/c1 first; x block 1 rides
        # the sync queue after its second W1 chunk so gate(1) never waits
        W1Q = [0, 1, 2, 0, 1, 2, 0, 1, 0, 1, 2, 0, 1, 2, 0, 1]
        for c in range(16):
            sl = slice(c * P, (c + 1) * P)
            QS[W1Q[c]].dma_start(W1s[:, sl, :], W1h[:, sl, :])
            if c == 3:
                load_xts(1, nc.sync)
        nc.scalar.dma_start(b2s[:], b2h[:])

        def load_w2():
            # W2 is not consumed until L2(0) (~72us) but 4MB of it racing the
            # prologue starves the HBM pipe (~330GB/s/core aggregate) right
            # when W1/x/bc0 are critical. Pace it: a tiny ACT write into one
            # element of every W2 chunk region (emitted after gelu ejc=11 of
            # block 0, ~28us) gives each W2 DMA a WAW dependency on the ACT,
            # so the transfers only start once L1(0) is mostly fed.
            nc.scalar.activation(
                W2s[0:1, :, 0:1], b1s[0:1, 0:NEJ], AF.Copy, scale=0.0
            )
            for g in range(NEJ):
                QS[g % 3].dma_start(W2s[:, g : g + 1, :], W2h[:, g : g + 1, :])
            load_xts(2, nc.sync)
            load_xts(3, nc.scalar)

        def gate_stage(blk):
            # gate logits (transposed): gt[e, t]; exp kept UNNORMALIZED.
            xts = xtss[blk]
            gt = ps_g.tile([E, T], FP, name=f"gt{blk}", tag="gt")
            for dc in range(DC):
                nc.tensor.matmul(
                    gt[:],
                    Wgs[:, dc, :],
                    xts[:, dc, :],
                    start=(dc == 0),
                    stop=(dc == DC - 1),
                )
            expv = epool.tile([E, T], BF, name=f"exp{blk}", tag="exp")
            nc.scalar.activation(expv[:], gt[:], AF.Exp, bias=bgs[:, 0:1])
            # broadcast exp rows across partitions for the hidden scaling
            # (partition-stride-0 DMA only legal from DRAM -> bounce there)
            expd = dpool.tile([E, T], BF, name=f"expd{blk}", tag="expd")
            nc.gpsimd.dma_start(expd[:], expv[:])
            bc = bcpool.tile([P, E, T], BF, name=f"bc{blk}", tag="bc")
            for e in range(E):
                nc.gpsimd.dma_start(bc[:, e, :], expd[e : e + 1, :].to_broadcast((P, T)))
            return expv, bc

        def l1_stage(blk, bc):
            xts = xtss[blk]
            sh = shpool.tile([P, NEJ, T], BF, name=f"sh{blk}", tag="sh")
            for ejc in range(NEJ):
                ht = ps_h.tile([P, T], FP, name=f"ht{blk}_{ejc}", tag="ht")
                for dc in range(DC):
                    nc.tensor.matmul(
                        ht[:],
                        W1s[:, ejc * P : (ejc + 1) * P, dc],
                        xts[:, dc, :],
                        start=(dc == 0),
                        stop=(dc == DC - 1),
                    )
                nc.scalar.activation(
                    sh[:, ejc, :], ht[:], AF.Gelu, bias=b1s[:, ejc : ejc + 1]
                )
                nc.vector.tensor_tensor(
                    sh[:, ejc, :],
                    sh[:, ejc, :],
                    bc[:, ejc // JC_PER_E, :],
                    mybir.AluOpType.mult,
                )
                if blk == 0 and ejc == 11:
                    load_w2()
            return sh

        OUT_ENG = [nc.sync, nc.scalar, nc.gpsimd]

        def l2_stage(blk, sh, expv):
            t0 = blk * T
            # per-token softmax denominators (landed in token-partition
            # layout via K=8 ones matmuls), then reciprocals on the DVE
            rcp = rpool.tile([P, TS], FP, name=f"rcp{blk}", tag="rcp")
            for ts in range(TS):
                s = ps_s.tile([P, 1], FP, name=f"s{blk}_{ts}", tag="s")
                nc.tensor.matmul(
                    s[:],
                    expv[:, ts * P : (ts + 1) * P],
                    ones8[:],
                    start=True,
                    stop=True,
                )
                nc.vector.reciprocal(rcp[:, ts : ts + 1], s[:])
            for ts in range(TS):
                tsl = slice(ts * P, (ts + 1) * P)
                for half in range(2):
                    o0 = half * OH
                    ops = ps_o.tile([P, OH], FP, name=f"ops{blk}_{ts}_{half}", tag="ops")
                    nc.tensor.matmul(
                        ops[:],
                        expv[:, tsl],
                        b2s[:, o0 : o0 + OH],
                        start=True,
                        stop=False,
                    )
                    for ejc in range(NEJ):
                        nc.tensor.matmul(
                            ops[:],
                            sh[:, ejc, tsl],
                            W2s[:, ejc, o0 : o0 + OH],
                            start=False,
                            stop=(ejc == NEJ - 1),
                        )
                    outsb = opool.tile([P, OH], BF, name=f"o{blk}_{ts}_{half}", tag="o")
                    nc.scalar.activation(
                        outsb[:], ops[:], AF.Copy, scale=rcp[:, ts : ts + 1]
                    )
                    OUT_ENG[(ts * 2 + half) % 3].dma_start(
                        out[t0 + ts * P : t0 + (ts + 1) * P, o0 : o0 + OH], outsb[:]
                    )

        # --- pipeline: gate(b+1) and L2(b-1) interleave with L1(b) -------
        states = {0: gate_stage(0)}
        sh_prev = None
        for blk in range(NB):
            sh = l1_stage(blk, states[blk][1])
            if blk + 1 < NB:
                states[blk + 1] = gate_stage(blk + 1)
            if blk >= 1:
                l2_stage(blk - 1, sh_prev, states[blk - 1][0])
            sh_prev = sh
        l2_stage(NB - 1, sh_prev, states[NB - 1][0])
    return nc


_CACHE = {}


def kernel(**inputs):
    x = np.asarray(inputs["x"], dtype=np.float32)
    W1 = np.asarray(inputs["W1"], dtype=np.float32)
    b1 = np.asarray(inputs["b1"], dtype=np.float32)
    W2 = np.asarray(inputs["W2"], dtype=np.float32)
    b2 = np.asarray(inputs["b2"], dtype=np.float32)
    Wg = np.asarray(inputs["Wg"], dtype=np.float32)
    bg = np.asarray(inputs["bg"], dtype=np.float32)

    # host pre-layouts (all partition-major, bf16 where a matmul consumes it)
    W1p = W1.transpose(1, 0, 2).reshape(D, EJ)  # [d, ej]
    W1h = np.ascontiguousarray(
        W1p.reshape(DC, P, EJ).transpose(1, 2, 0)
    ).astype(BF_NP)  # [p, ej, dc]
    Wgh = np.ascontiguousarray(Wg.reshape(DC, P, E).transpose(1, 0, 2)).astype(BF_NP)
    W2h = np.ascontiguousarray(
        W2.reshape(EJ, O).reshape(NEJ, P, O).transpose(1, 0, 2)
    ).astype(BF_NP)  # [p, ec, o]
    b1h = np.ascontiguousarray(b1.reshape(EJ).reshape(NEJ, P).T)
    bgh = np.ascontiguousarray(bg.reshape(E, 1))
    b2h = b2.astype(BF_NP)

    if "nc" not in _CACHE:
        _CACHE["nc"] = _build_nc()
    nc = _CACHE["nc"]

    x16 = x.astype(BF_NP)
    in_maps = []
    for c in range(NCORES):
        xs = x16[c * NTOK : (c + 1) * NTOK]  # [ntok, d]
        xhc = np.ascontiguousarray(
            xs.T.reshape(DC, P, NTOK).transpose(1, 0, 2)
        )  # [p, dc, t]
        in_maps.append(
            {
                "xh": xhc,
                "W1h": W1h,
                "Wgh": Wgh,
                "W2h": W2h,
                "b1h": b1h,
                "bgh": bgh,
                "b2h": b2h,
            }
        )

    res = run_bass_kernel_spmd(nc, in_maps, list(range(NCORES)))
    kernel.last = res
    return np.concatenate(
        [res.results[c]["out"].astype(np.float32) for c in range(NCORES)], axis=0
    )
